# revision 23
# baseline (speedup 1.0000x reference)
"""Multihead attention (B=4, S=2048, E=1024, H=16, D=64) on 8 Trainium2 cores.

Sharding: core c = (batch b = c//2, head-half hh = c%2). Each core computes one
batch's attention for 8 heads (512 of the 1024 projection columns), producing a
partial output (row-split Wo); the host sums the two partials per batch.

v3 design (vs the 443us v2):
- Phase B emits matmuls in "duo" blocks (2 st-tiles): 4 score MMs then 4 attnV
  MMs (lagged 2 duos). Long same-class PE runs let LDWEIGHTS hide in the
  background weight buffer; the v2 per-st interleave paid ~110ns per
  stationary-class transition (measured 318/335ns MMs vs the 216ns pitch).
- Scores for a duo land in one [128,2,512] PSUM pair-tile, so exp runs as one
  engine op over 1024 free elements, amortizing the fixed per-op overhead
  (ScalarE (172+1024)/1.2 = 997ns per 2 tiles vs 2x720; DVE similar).
- exp split: ScalarE true-exp for head A, DVE Schraudolph bit-trick for head B.
  On duos 1/4/6 ScalarE covers head B too while the DVE runs one stage of the
  previous group's 1/den Newton chain, so the DVE never delays the PSUM-tile
  recycle. (Alternatives tried: ScalarE exp(-ln(den)) thrashes activation
  tables - the greedy set-picker alternates exp_and_others/natural_log, 35
  loads x 2.7us; the custom-DVE reciprocal_approx_fast op silently does not
  execute on the axon PJRT path - output buffer left stale.)
- Phase A k-projection accumulates kc-outer over 8 live PSUM banks with
  per-chunk weight/x DMAs, so the first matmul starts ~2us in (v2 waited ~20us
  for the full 5MB x+w load).
- Phase C runs in 2-et waves with the t=3 (last-normalized pair) accumulation
  emitted last, covering the chunk-end den/norm latency.
"""
import sys

sys.path.insert(0, "/opt/trn_rl_repo")

import numpy as np

import concourse.bacc as bacc
import concourse.mybir as mybir
import concourse.tile as tile
from concourse.bass_utils import run_bass_kernel_spmd

E = 1024
H = 16
D = 64
B = 4
S = 2048
HH = E // 2          # projection cols per core
N_CORES = 8
P = 128
NCH = 4              # sq-chunks of 512
CH = 512
NDUO = 8             # st-duos per (pair, chunk) group
LAG = 2              # attnV trails scores by this many duos
f32 = mybir.dt.float32
f16 = mybir.dt.float16
i16 = mybir.dt.int16
AF = mybir.ActivationFunctionType
ALU = mybir.AluOpType

# Schraudolph fast exp on DVE: exp(s*0.125) ~= bitcast_f16(int16(s*SCH_A + SCH_B))
SCH_A = 0.125 * 1024.0 / float(np.log(2.0))   # 184.664
SCH_B = 15360.0 - 44.0
# fp32 bit-trick reciprocal seed (refined by one Newton step on DVE)
RMAGIC = 0x7EF311C3

_cached = {}


def _build():
    nc = bacc.Bacc(None, target_bir_lowering=False)

    xqT = nc.declare_dram_parameter("xqT", [E, S], f16, isOutput=False)
    xkT = nc.declare_dram_parameter("xkT", [E, S], f16, isOutput=False)
    xvT = nc.declare_dram_parameter("xvT", [E, S], f16, isOutput=False)
    wq = nc.declare_dram_parameter("wq", [P, 8, HH], f16, isOutput=False)
    wk = nc.declare_dram_parameter("wk", [P, 8, HH], f16, isOutput=False)
    wv = nc.declare_dram_parameter("wv", [P, 8, HH], f16, isOutput=False)
    bq_col = nc.declare_dram_parameter("bq_col", [P, 4], f32, isOutput=False)
    bk_col = nc.declare_dram_parameter("bk_col", [P, 4], f32, isOutput=False)
    bv_bc = nc.declare_dram_parameter("bv_bc", [P, 8, D], f16, isOutput=False)
    wo = nc.declare_dram_parameter("wo", [P, 4, E], f16, isOutput=False)
    bo_col = nc.declare_dram_parameter("bo_col", [P, 8], f32, isOutput=False)
    yT = nc.declare_dram_parameter("yT", [E, S], f16, isOutput=True)

    from contextlib import ExitStack

    with tile.TileContext(nc) as tc, ExitStack() as stack:
        main = stack.enter_context(tc.tile_pool(name="main", bufs=1))
        qT = main.tile([P, 4, S], f16)      # [d-in-pair, pair, sq]
        kT = main.tile([P, 4, S], f16)
        vbuf = main.tile([P, 16, 8, D + 1], f16)  # [sv, s-tile, head, d|1]
        ou = main.tile([P, 4, S], f16)      # normalized attention out
        wo_t = main.tile([P, 4, E], f16)
        bqc = main.tile([P, 4], f32)
        bkc = main.tile([P, 4], f32)
        boc = main.tile([P, 8], f32)
        bvt = main.tile([P, 8, D], f16)
        warm = main.tile([1, 8], f32)

        warm2 = main.tile([2, 8], f16)
        nc.vector.memset(vbuf[:, :, :, D], 1.0)
        nc.vector.memset(warm[:], 1.0)
        nc.vector.memset(warm2[:], 1.0)
        # Preload the exp activation-table set during Phase A's DMA window.
        nc.scalar.activation(warm[0:1, 0:4], warm[0:1, 0:4], AF.Exp)

        # ---------------- Phase A: projections (x comes in pre-transposed) ----
        # k first (kc-outer over 8 live PSUM banks so the PE starts on the
        # first 128KB w + 256KB x instead of the full 5MB), then q, then v.
        # The kc=0 operands ride the sync queue, which boots ~6us before the
        # gpsimd queue that carries the bulk x transfers.
        with tc.tile_pool(name="wp", bufs=2) as wp, \
             tc.tile_pool(name="xp", bufs=2) as xp:
            wk_t = wp.tile([P, 8, HH], f16, tag="w", name="w_k")
            xk_t = xp.tile([P, 8, S], f16, tag="x", name="x_k")
            nc.sync.dma_start(out=wk_t[:, 0, :], in_=wk[:, 0, :])
            nc.sync.dma_start(out=xk_t[:, 0, 0:2 * CH], in_=xkT[0:P, 0:2 * CH])
            nc.sync.dma_start(out=wk_t[:, 1, :], in_=wk[:, 1, :])
            nc.sync.dma_start(out=xk_t[:, 1, 0:2 * CH],
                              in_=xkT[P:2 * P, 0:2 * CH])
            nc.sync.dma_start(out=wk_t[:, 2, :], in_=wk[:, 2, :])
            nc.sync.dma_start(out=xk_t[:, 2, 0:2 * CH],
                              in_=xkT[2 * P:3 * P, 0:2 * CH])
            for kc in range(3, 8):
                nc.sync.dma_start(out=wk_t[:, kc, :], in_=wk[:, kc, :])
            nc.sync.dma_start(out=bqc[:], in_=bq_col[:])
            nc.sync.dma_start(out=bkc[:], in_=bk_col[:])
            nc.sync.dma_start(out=bvt[:], in_=bv_bc[:])
            for half in range(2):
                hs = slice(half * 2 * CH, (half + 1) * 2 * CH)
                for kc in range(8):
                    if kc <= 2 and half == 0:
                        continue
                    nc.gpsimd.dma_start(out=xk_t[:, kc, hs],
                                        in_=xkT[kc * P:(kc + 1) * P, hs])
            # Warm the gpsimd attn ucode library (partition_broadcast) after
            # the k DMA pushes: the ~7us library reload otherwise stalls
            # either the x stream (if first) or every engine at the first
            # Phase B broadcast (if left to Phase B).
            nc.gpsimd.partition_broadcast(warm2[:, :], warm2[0:1, :])

            with tc.tile_pool(name="ps_a", bufs=8, space="PSUM") as ps_a:
                def proj_qk(w_t, x_t, dest, bcol, kind):
                    # kc-outer over 8 live PSUM banks; bias adds split across
                    # ScalarE/DVE so each half's drain stays off the PE path
                    for half in range(2):
                        ug = [(u, half * 2 + g)
                              for u in range(4) for g in range(2)]
                        tk = [ps_a.tile([P, CH], f32, tag="pj",
                                        name=f"p{kind}{half}_{i}")
                              for i in range(8)]
                        for kc in range(8):
                            for i, (u, g) in enumerate(ug):
                                nc.tensor.matmul(tk[i][:],
                                                 lhsT=w_t[:, kc, u * P:(u + 1) * P],
                                                 rhs=x_t[:, kc, g * CH:(g + 1) * CH],
                                                 start=(kc == 0), stop=(kc == 7))
                        for i, (u, g) in enumerate(ug):
                            dst = dest[:, u, g * CH:(g + 1) * CH]
                            if i % 2 == 0:
                                nc.scalar.add(dst, tk[i][:], bcol[:, u:u + 1])
                            else:
                                nc.vector.tensor_scalar_add(dst, tk[i][:],
                                                            bcol[:, u:u + 1])

                proj_qk(wk_t, xk_t, kT, bkc, "k")

                # q and v: x DMA overlaps the k matmuls
                wq_t = wp.tile([P, 8, HH], f16, tag="w", name="w_q")
                nc.sync.dma_start(out=wq_t[:], in_=wq[:])
                xq_t = xp.tile([P, 8, S], f16, tag="x", name="x_q")
                for kc in range(8):
                    nc.gpsimd.dma_start(out=xq_t[:, kc, :],
                                        in_=xqT[kc * P:(kc + 1) * P, :])
                proj_qk(wq_t, xq_t, qT, bqc, "q")

                wv_t = wp.tile([P, 8, HH], f16, tag="w", name="w_v")
                nc.sync.dma_start(out=wv_t[:], in_=wv[:])
                xv_t = xp.tile([P, 8, S], f16, tag="x", name="x_v")
                for kc in range(8):
                    nc.gpsimd.dma_start(out=xv_t[:, kc, :],
                                        in_=xvT[kc * P:(kc + 1) * P, :])
                for sv in range(16):
                    pp = ps_a.tile([P, 8, D], f32, tag="pj", name=f"pj_v{sv}")
                    for kc in range(8):
                        nc.tensor.matmul(pp[:],
                                         lhsT=xv_t[:, kc, sv * P:(sv + 1) * P],
                                         rhs=wv_t[:, kc, :],
                                         start=(kc == 0), stop=(kc == 7))
                    nc.vector.tensor_add(vbuf[:, sv, :, 0:D], pp[:], bvt[:])

        nc.sync.dma_start(out=boc[:], in_=bo_col[:])
        nc.sync.dma_start(out=wo_t[:], in_=wo[:])

        # ---------------- Phase B: attention; Phase C: out-proj per chunk ----
        with tc.tile_pool(name="ex", bufs=1) as ep, \
             tc.tile_pool(name="ivp", bufs=1) as ivp, \
             tc.tile_pool(name="otp", bufs=3) as otp, \
             tc.tile_pool(name="ps_s", bufs=1, space="PSUM") as ps_s, \
             tc.tile_pool(name="ps_o", bufs=2, space="PSUM") as ps_o:
            # Deferred -1/den (3-stage Newton, one [1,1024] DVE op per stage,
            # spread across the next group's duo slots) -> gpsimd broadcast ->
            # ou multiplies. At most one group is in flight at a time.
            den_pend = []

            def den_step(p):
                st = p["stage"]
                p["stage"] += 1
                fpso = p["pso"]
                if st == 0:
                    # bit-trick seed for -1/den
                    p["sd"] = ivp.tile([1, 2, CH], f32, tag="sd", bufs=2, name="sd")
                    nc.vector.tensor_scalar(
                        out=p["sd"][:].bitcast(mybir.dt.int32),
                        in0=fpso[D:D + 1, :, :].bitcast(mybir.dt.int32),
                        scalar1=-1, scalar2=RMAGIC,
                        op0=ALU.mult, op1=ALU.add)
                elif st == 1:
                    p["tt"] = ivp.tile([1, 2, CH], f32, tag="tt", bufs=2, name="tt")
                    nc.vector.tensor_mul(p["tt"][:], p["sd"][:],
                                         fpso[D:D + 1, :, :])
                elif st == 2:
                    # one Newton step; sign folded into Wo on the host
                    p["inv"] = ivp.tile([1, 2, CH], f16, tag="iv", bufs=2, name="inv")
                    nc.vector.scalar_tensor_tensor(
                        out=p["inv"][:], in0=p["tt"][:], scalar=2.0,
                        in1=p["sd"][:], op0=ALU.subtract, op1=ALU.mult)
                elif st == 3:
                    p["bc"] = ivp.tile([D, 2, CH], f16, tag="bc", bufs=2, name="bc")
                    nc.gpsimd.partition_broadcast(p["bc"][:, :, :],
                                                  p["inv"][0:1, :, :])
                else:
                    nc.vector.tensor_mul(ou[0:D, p["pr"], p["cs"]],
                                         fpso[0:D, 0, :], p["bc"][:, 0, :])
                    nc.vector.tensor_mul(ou[D:2 * D, p["pr"], p["cs"]],
                                         fpso[0:D, 1, :], p["bc"][:, 1, :])

            def den_hook(stage_limit):
                while den_pend and den_pend[0]["stage"] < stage_limit:
                    den_step(den_pend[0])

            def den_drain():
                while den_pend:
                    p = den_pend[0]
                    while p["stage"] <= 4:
                        den_step(p)
                    den_pend.pop(0)

            def emit_phase_c(c):
                # out-projection for chunk c in 2 waves of 4 ets (each po
                # tile spans 2 PSUM banks and holds 2 et outputs)
                cs = slice(c * CH, (c + 1) * CH)
                for w in range(2):
                    po = []
                    for i in range(2):
                        po.append(ps_s.tile([P, 2, CH], f32,
                                            tag=("sa" if i == 0 else "sb"),
                                            bufs=1, name=f"po{c}_{w}_{i}"))
                    for t in range(4):
                        for i in range(2):
                            for jj in range(2):
                                et = 4 * w + 2 * i + jj
                                nc.tensor.matmul(
                                    po[i][:, jj, :],
                                    lhsT=wo_t[:, t, et * P:(et + 1) * P],
                                    rhs=ou[:, t, cs],
                                    start=(t == 0), stop=(t == 3))
                    for i in range(2):
                        for jj in range(2):
                            et = 4 * w + 2 * i + jj
                            out_t = otp.tile([P, CH], f16, tag="ot", bufs=4,
                                             name=f"ot{c}_{et}")
                            if (i + jj) % 2 == 0:
                                nc.scalar.add(out_t[:], po[i][:, jj, :],
                                              boc[:, et:et + 1])
                            else:
                                nc.vector.tensor_scalar_add(out_t[:],
                                                            po[i][:, jj, :],
                                                            boc[:, et:et + 1])
                            nc.sync.dma_start(out=yT[et * P:(et + 1) * P, cs],
                                              in_=out_t[:])

            for c in range(NCH):
                cs = slice(c * CH, (c + 1) * CH)
                for pr in range(4):
                    hA, hB = 2 * pr, 2 * pr + 1
                    pso = ps_o.tile([D + 1, 2, CH], f32, tag="o", bufs=2,
                                    name=f"pso{c}{pr}")
                    vpend = []
                    for d in range(NDUO):
                        # both heads of one st share a PSUM tile so a single
                        # exp op releases it (a split releaser desynchronizes
                        # the A/B score pairs in the scheduler)
                        pst2 = []
                        for j in range(2):
                            ps2 = ps_s.tile([P, 2, CH], f32,
                                            tag=("sa" if j == 0 else "sb"),
                                            bufs=1, name=f"ps{c}{pr}{d}{j}")
                            ks = slice((2 * d + j) * P, (2 * d + j + 1) * P)
                            nc.tensor.matmul(ps2[:, 0, :], lhsT=kT[0:D, pr, ks],
                                             rhs=qT[0:D, pr, cs],
                                             start=True, stop=True)
                            nc.tensor.matmul(ps2[:, 1, :], lhsT=kT[D:P, pr, ks],
                                             rhs=qT[D:P, pr, cs],
                                             start=True, stop=True)
                            pst2.append(ps2)
                        # exp per st (1024 free elems per op, both heads)
                        # DVE (slower op) takes the even st whose scores
                        # finish first, ScalarE the odd st: both PSUM tiles
                        # then free at ~the same time, so the next duo's four
                        # score MMs dispatch as one run (split releases cost a
                        # stationary-class transition per st).
                        ex2 = []
                        for j in range(2):
                            ex = ep.tile([P, 2, CH], f16,
                                         tag=("xa" if j == 0 else "xb"), bufs=4,
                                         name=f"ex{c}{pr}{d}{j}")
                            if j == 1 or d in (0, 2, 4):
                                # ScalarE covers the even st too while the DVE
                                # runs one Newton stage for the prev group
                                nc.scalar.activation(ex[:], pst2[j][:], AF.Exp,
                                                     scale=0.125)
                            else:
                                nc.vector.tensor_scalar(out=ex[:].bitcast(i16),
                                                        in0=pst2[j][:],
                                                        scalar1=SCH_A,
                                                        scalar2=SCH_B,
                                                        op0=ALU.mult,
                                                        op1=ALU.add)
                            ex2.append(ex)
                        vpend.append((d, ex2[0], ex2[1]))
                        if d == 0:
                            den_hook(1)      # Newton seed for prev group
                        elif d == 2:
                            den_hook(2)      # Newton multiply
                        elif d == 4:
                            den_hook(4)      # Newton refine + gpsimd broadcast
                        elif d == 6:
                            den_hook(5)      # ou multiplies: early enough that
                                             # Phase C's t=3 never waits
                        if len(vpend) > LAG:
                            dd, e0, e1 = vpend.pop(0)
                            for j, ee in ((0, e0), (1, e1)):
                                pst = 2 * dd + j
                                nc.tensor.matmul(pso[:, 0, :],
                                                 lhsT=vbuf[:, pst, hA, :],
                                                 rhs=ee[:, 0, :],
                                                 start=(pst == 0), stop=False,
                                                 skip_group_check=True)
                                nc.tensor.matmul(pso[:, 1, :],
                                                 lhsT=vbuf[:, pst, hB, :],
                                                 rhs=ee[:, 1, :],
                                                 start=(pst == 0), stop=False,
                                                 skip_group_check=True)
                    for dd, e0, e1 in vpend:
                        for j, ee in ((0, e0), (1, e1)):
                            pst = 2 * dd + j
                            nc.tensor.matmul(pso[:, 0, :],
                                             lhsT=vbuf[:, pst, hA, :],
                                             rhs=ee[:, 0, :],
                                             start=False, stop=(pst == 15),
                                             skip_group_check=True)
                            nc.tensor.matmul(pso[:, 1, :],
                                             lhsT=vbuf[:, pst, hB, :],
                                             rhs=ee[:, 1, :],
                                             start=False, stop=(pst == 15),
                                             skip_group_check=True)
                    # the previous group's ou multiplies land after this
                    # group's Schraudolph stream so the DVE never delays exp
                    den_drain()
                    den_pend.append({"stage": 0, "pso": pso, "pr": pr, "cs": cs})
                    if c > 0 and pr == 0:
                        # chunk c-1's out-projection waited one full group, so
                        # its pair-3 normalization is already complete
                        emit_phase_c(c - 1)
            den_drain()
            emit_phase_c(NCH - 1)

    nc.finalize()
    return nc


def _get_nc():
    if "nc" not in _cached:
        _cached["nc"] = _build()
    return _cached["nc"]


def _in_maps(query, key, value, Wq, bq, Wk, bk, Wv, bv, Wo, bo):
    query = np.asarray(query, np.float32)
    key = np.asarray(key, np.float32)
    value = np.asarray(value, np.float32)
    maps = []
    xT = {}
    for b in range(B):
        xT[("q", b)] = np.ascontiguousarray(query[b].T.astype(np.float16))
        xT[("k", b)] = np.ascontiguousarray(key[b].T.astype(np.float16))
        xT[("v", b)] = np.ascontiguousarray(value[b].T.astype(np.float16))
    for c in range(N_CORES):
        b, hh = divmod(c, 2)
        sl = slice(hh * HH, (hh + 1) * HH)

        def wcols(W):
            Ws = np.asarray(W, np.float32)[:, sl].astype(np.float16)
            return np.ascontiguousarray(Ws.reshape(8, P, HH).transpose(1, 0, 2))

        # negated: the kernel's normalization produces -attn (3-op Newton
        # yields -1/den), so -Wo restores the sign in the output projection
        wo_s = (-np.asarray(Wo, np.float32)[sl, :]).astype(np.float16)   # [512, E]
        wo_r = np.ascontiguousarray(wo_s.reshape(4, P, E).transpose(1, 0, 2))
        bo_c = (np.asarray(bo, np.float32).reshape(8, P).T if hh == 0
                else np.zeros((P, 8), np.float32))
        bv_b = np.ascontiguousarray(
            np.tile(np.asarray(bv, np.float32)[sl].astype(np.float16),
                    (P, 1)).reshape(P, 8, D))
        maps.append({
            "xqT": xT[("q", b)],
            "xkT": xT[("k", b)],
            "xvT": xT[("v", b)],
            "wq": wcols(Wq),
            "wk": wcols(Wk),
            "wv": wcols(Wv),
            "bq_col": np.ascontiguousarray(np.asarray(bq, np.float32)[sl].reshape(4, P).T),
            "bk_col": np.ascontiguousarray(np.asarray(bk, np.float32)[sl].reshape(4, P).T),
            "bv_bc": bv_b,
            "wo": wo_r,
            "bo_col": np.ascontiguousarray(bo_c),
        })
    return maps


def _assemble(results):
    outs = [results[c]["yT"] for c in range(N_CORES)]
    return np.stack([
        (outs[2 * b].astype(np.float32) + outs[2 * b + 1].astype(np.float32)).T
        for b in range(B)
    ]).astype(np.float32)


def kernel(**inputs):
    nc = _get_nc()
    maps = _in_maps(**inputs)
    r = run_bass_kernel_spmd(nc, maps, list(range(N_CORES)))
    return _assemble(r.results)


def _ensure_ntff_hook():
    """Register the axon NTFF profiling hook (missing antenv.axon_hooks shim)."""
    import contextlib
    import ctypes
    import types

    try:
        from antenv.axon_hooks import get_axon_ntff_profile_hook
        if get_axon_ntff_profile_hook() is not None:
            return
    except ImportError:
        pass

    import antenv

    holder = {}
    mod = types.ModuleType("antenv.axon_hooks")
    mod.set_axon_ntff_profile_hook = lambda h: holder.__setitem__("h", h)
    mod.get_axon_ntff_profile_hook = lambda: holder.get("h")
    sys.modules["antenv.axon_hooks"] = mod
    antenv.axon_hooks = mod

    so_path = "/opt/axon/libaxon_pjrt.so"
    lib = ctypes.CDLL(so_path)
    if not hasattr(lib, "axon_start_nrt_profile"):
        return
    lib.axon_start_nrt_profile.argtypes = [ctypes.POINTER(ctypes.c_int64), ctypes.c_size_t]
    lib.axon_start_nrt_profile.restype = ctypes.c_int64
    lib.axon_stop_nrt_profile.argtypes = [ctypes.c_char_p]
    lib.axon_stop_nrt_profile.restype = ctypes.c_int64

    @contextlib.contextmanager
    def _hook(output_dir, device_ids):
        import jax

        jax.devices()
        if device_ids:
            ids = (ctypes.c_int64 * len(device_ids))(*device_ids)
            rc = lib.axon_start_nrt_profile(ids, len(device_ids))
        else:
            rc = lib.axon_start_nrt_profile(None, 0)
        if rc != 0:
            raise RuntimeError(f"axon_start_nrt_profile rc={rc}")
        try:
            yield
        finally:
            n = lib.axon_stop_nrt_profile(str(output_dir).encode())
            if n < 0:
                raise RuntimeError(f"axon_stop_nrt_profile rc={n}")

    mod.set_axon_ntff_profile_hook(_hook)


def kernel_traced(tmpdir=None, **inputs):
    """Like kernel() but with NTFF tracing; returns (output, exec_time_ns)."""
    _ensure_ntff_hook()
    import concourse.bass_utils as bu
    bu.upload_artifacts = lambda d: d  # no artifact bucket in this container
    nc = _get_nc()
    maps = _in_maps(**inputs)
    r = run_bass_kernel_spmd(nc, maps, list(range(N_CORES)), trace=True, tmpdir=tmpdir)
    return _assemble(r.results), r.exec_time_ns


# revision 33
# speedup vs baseline: 1.1859x; 1.1859x over previous
"""Multihead attention (B=4, S=2048, E=1024, H=16, D=64) on 8 Trainium2 cores.

Sharding: core c = (batch b = c//2, head-half hh = c%2). Each core computes one
batch's attention for 8 heads (512 of the 1024 projection columns), producing a
partial output (row-split Wo); the host sums the two partials per batch.

v3 design (~405us vs the 443us v2; PE-bound at ~85% occupancy):
- Phase B emits matmuls in "duo" blocks (2 st-tiles): 4 score MMs, 2 duo-wide
  exp ops, then the attnV block lagged LAG=3 duos. Both heads of one st share
  a [128,2,512] PSUM tile so a single exp op releases it; the DVE (slower op)
  takes the even st whose scores finish first and ScalarE the odd st, making
  the two releases converge so the next duo's scores dispatch as one PE run
  (staggered releases cost a ~110-130ns stationary-class transition per st).
- exp: ScalarE true-exp, DVE Schraudolph bit-trick (~31% of tiles); on duos
  0/2/4 ScalarE covers both sts while the DVE runs one stage of the previous
  group's 1/den Newton chain ([1,1024] single-lane ops, so they must never
  displace a Schraudolph op whose PSUM tile gates the score pipeline).
- 1/den: 3-op Newton bit-trick spread across the NEXT group's duo slots
  (seed@d0, mult@d2, refine@d4, gpsimd broadcast@d5, ou muls@d7), freeing the
  double-buffered pso accumulator most of a group before it is reused.
  (Alternatives that failed: ScalarE exp(-ln(den)) thrashes activation-table
  sets - the greedy picker alternates exp_and_others/natural_log at 2.7us per
  load; custom-DVE reciprocal_approx_fast silently no-ops on the axon PJRT
  path; merging all four duo scores into one 4-bank PSUM tile serializes the
  two exp engines - shared-tile accessors are sequenced, +1.3us/duo.)
- Phase A: k/q projections run kc-outer over 8 live PSUM banks in one shared
  pool (separate pools cost a ~3.3us drain per transition); the first chunks
  of w and x ride the sync DMA queue, which boots ~6us before the gpsimd
  queue carrying the bulk x stream; bias adds alternate ScalarE/DVE.
- The gpsimd attn ucode library (partition_broadcast) is warmed right after
  the x DMA pushes: the ~7us runtime reload otherwise stalls every engine at
  the first Phase B broadcast.
- Phase C(c) is emitted after chunk c+1's FIRST group, so the pair-3
  normalization chain completes during that group; it runs as 2 waves of 4
  ets with the t=3 accumulation last.
"""
import sys

sys.path.insert(0, "/opt/trn_rl_repo")

import numpy as np

import concourse.bacc as bacc
import concourse.mybir as mybir
import concourse.tile as tile
from concourse.bass_utils import run_bass_kernel_spmd

E = 1024
H = 16
D = 64
B = 4
S = 2048
HH = E // 2          # projection cols per core
N_CORES = 8
P = 128
NCH = 4              # sq-chunks of 512
CH = 512
NDUO = 8             # st-duos per (pair, chunk) group
LAG = 3              # attnV trails scores by this many duos
f32 = mybir.dt.float32
f16 = mybir.dt.float16
i16 = mybir.dt.int16
AF = mybir.ActivationFunctionType
ALU = mybir.AluOpType

# Schraudolph fast exp on DVE: exp(s*0.125) ~= bitcast_f16(int16(s*SCH_A + SCH_B))
SCH_A = 0.125 * 1024.0 / float(np.log(2.0))   # 184.664
SCH_B = 15360.0 - 44.0
# fp32 bit-trick reciprocal seed (refined by one Newton step on DVE)
RMAGIC = 0x7EF311C3

_cached = {}


def _build():
    nc = bacc.Bacc(None, target_bir_lowering=False)

    xqT = nc.declare_dram_parameter("xqT", [E, S], f16, isOutput=False)
    xkT = nc.declare_dram_parameter("xkT", [E, S], f16, isOutput=False)
    xvT = nc.declare_dram_parameter("xvT", [E, S], f16, isOutput=False)
    wq = nc.declare_dram_parameter("wq", [P, 8, HH], f16, isOutput=False)
    wk = nc.declare_dram_parameter("wk", [P, 8, HH], f16, isOutput=False)
    wv = nc.declare_dram_parameter("wv", [P, 8, HH], f16, isOutput=False)
    bq_col = nc.declare_dram_parameter("bq_col", [P, 4], f32, isOutput=False)
    bk_col = nc.declare_dram_parameter("bk_col", [P, 4], f32, isOutput=False)
    bv_bc = nc.declare_dram_parameter("bv_bc", [P, 8, D], f16, isOutput=False)
    wo = nc.declare_dram_parameter("wo", [P, 4, E], f16, isOutput=False)
    bo_col = nc.declare_dram_parameter("bo_col", [P, 8], f32, isOutput=False)
    yT = nc.declare_dram_parameter("yT", [E, S], f16, isOutput=True)

    from contextlib import ExitStack

    with tile.TileContext(nc) as tc, ExitStack() as stack:
        main = stack.enter_context(tc.tile_pool(name="main", bufs=1))
        qT = main.tile([P, 4, S], f16)      # [d-in-pair, pair, sq]
        kT = main.tile([P, 4, S], f16)
        vbuf = main.tile([P, 16, 8, D + 1], f16)  # [sv, s-tile, head, d|1]
        ou = main.tile([P, 4, S], f16)      # normalized attention out
        wo_t = main.tile([P, 4, E], f16)
        bqc = main.tile([P, 4], f32)
        bkc = main.tile([P, 4], f32)
        boc = main.tile([P, 8], f32)
        bvt = main.tile([P, 8, D], f16)
        warm = main.tile([1, 8], f32)

        warm2 = main.tile([2, 8], f16)
        nc.vector.memset(vbuf[:, :, :, D], 1.0)
        nc.vector.memset(warm[:], 1.0)
        nc.vector.memset(warm2[:], 1.0)
        # Preload the exp activation-table set during Phase A's DMA window.
        nc.scalar.activation(warm[0:1, 0:4], warm[0:1, 0:4], AF.Exp)

        # ---------------- Phase A: projections (x comes in pre-transposed) ----
        # k first (kc-outer over 8 live PSUM banks so the PE starts on the
        # first 128KB w + 256KB x instead of the full 5MB), then q, then v.
        # The kc=0 operands ride the sync queue, which boots ~6us before the
        # gpsimd queue that carries the bulk x transfers.
        with tc.tile_pool(name="wp", bufs=2) as wp, \
             tc.tile_pool(name="xp", bufs=2) as xp:
            wk_t = wp.tile([P, 8, HH], f16, tag="w", name="w_k")
            xk_t = xp.tile([P, 8, S], f16, tag="x", name="x_k")
            nc.sync.dma_start(out=wk_t[:, 0, :], in_=wk[:, 0, :])
            nc.sync.dma_start(out=xk_t[:, 0, 0:2 * CH], in_=xkT[0:P, 0:2 * CH])
            nc.sync.dma_start(out=wk_t[:, 1, :], in_=wk[:, 1, :])
            nc.sync.dma_start(out=xk_t[:, 1, 0:2 * CH],
                              in_=xkT[P:2 * P, 0:2 * CH])
            nc.sync.dma_start(out=wk_t[:, 2, :], in_=wk[:, 2, :])
            nc.sync.dma_start(out=xk_t[:, 2, 0:2 * CH],
                              in_=xkT[2 * P:3 * P, 0:2 * CH])
            for kc in range(3, 8):
                nc.sync.dma_start(out=wk_t[:, kc, :], in_=wk[:, kc, :])
            nc.sync.dma_start(out=bqc[:], in_=bq_col[:])
            nc.sync.dma_start(out=bkc[:], in_=bk_col[:])
            nc.sync.dma_start(out=bvt[:], in_=bv_bc[:])
            for half in range(2):
                hs = slice(half * 2 * CH, (half + 1) * 2 * CH)
                for kc in range(8):
                    if kc <= 2 and half == 0:
                        continue
                    nc.gpsimd.dma_start(out=xk_t[:, kc, hs],
                                        in_=xkT[kc * P:(kc + 1) * P, hs])

            with tc.tile_pool(name="ps_a", bufs=8, space="PSUM") as ps_a:
                def proj_qk(w_t, x_t, dest, bcol, kind):
                    # kc-outer over 8 live PSUM banks; bias adds split across
                    # ScalarE/DVE so each half's drain stays off the PE path
                    for half in range(2):
                        ug = [(u, half * 2 + g)
                              for u in range(4) for g in range(2)]
                        tk = [ps_a.tile([P, CH], f32, tag="pj",
                                        name=f"p{kind}{half}_{i}")
                              for i in range(8)]
                        for kc in range(8):
                            for i, (u, g) in enumerate(ug):
                                nc.tensor.matmul(tk[i][:],
                                                 lhsT=w_t[:, kc, u * P:(u + 1) * P],
                                                 rhs=x_t[:, kc, g * CH:(g + 1) * CH],
                                                 start=(kc == 0), stop=(kc == 7))
                        for i, (u, g) in enumerate(ug):
                            dst = dest[:, u, g * CH:(g + 1) * CH]
                            if i % 2 == 0:
                                nc.scalar.add(dst, tk[i][:], bcol[:, u:u + 1])
                            else:
                                nc.vector.tensor_scalar_add(dst, tk[i][:],
                                                            bcol[:, u:u + 1])

                proj_qk(wk_t, xk_t, kT, bkc, "k")

                # q and v: x DMA overlaps the k matmuls
                wq_t = wp.tile([P, 8, HH], f16, tag="w", name="w_q")
                nc.sync.dma_start(out=wq_t[:], in_=wq[:])
                xq_t = xp.tile([P, 8, S], f16, tag="x", name="x_q")
                for half in range(2):
                    hs = slice(half * 2 * CH, (half + 1) * 2 * CH)
                    for kc in range(8):
                        nc.gpsimd.dma_start(out=xq_t[:, kc, hs],
                                            in_=xqT[kc * P:(kc + 1) * P, hs])
                proj_qk(wq_t, xq_t, qT, bqc, "q")

                wv_t = wp.tile([P, 8, HH], f16, tag="w", name="w_v")
                nc.sync.dma_start(out=wv_t[:], in_=wv[:])
                xv_t = xp.tile([P, 8, S], f16, tag="x", name="x_v")
                for half in range(2):
                    hs = slice(half * 2 * CH, (half + 1) * 2 * CH)
                    for kc in range(8):
                        nc.gpsimd.dma_start(out=xv_t[:, kc, hs],
                                            in_=xvT[kc * P:(kc + 1) * P, hs])
                # Warm the gpsimd attn ucode library (partition_broadcast)
                # once all x DMA pushes are queued: the ~7us library reload
                # otherwise stalls the x stream (if earlier) or every engine
                # at the first Phase B broadcast (if left to Phase B).
                nc.gpsimd.partition_broadcast(warm2[:, :], warm2[0:1, :])
                for sv in range(16):
                    pp = ps_a.tile([P, 8, D], f32, tag="pj", name=f"pj_v{sv}")
                    for kc in range(8):
                        nc.tensor.matmul(pp[:],
                                         lhsT=xv_t[:, kc, sv * P:(sv + 1) * P],
                                         rhs=wv_t[:, kc, :],
                                         start=(kc == 0), stop=(kc == 7))
                    nc.vector.tensor_add(vbuf[:, sv, :, 0:D], pp[:], bvt[:])

        nc.sync.dma_start(out=boc[:], in_=bo_col[:])
        nc.sync.dma_start(out=wo_t[:], in_=wo[:])

        # ---------------- Phase B: attention; Phase C: out-proj per chunk ----
        with tc.tile_pool(name="ex", bufs=1) as ep, \
             tc.tile_pool(name="ivp", bufs=1) as ivp, \
             tc.tile_pool(name="otp", bufs=3) as otp, \
             tc.tile_pool(name="ps_s", bufs=1, space="PSUM") as ps_s, \
             tc.tile_pool(name="ps_o", bufs=2, space="PSUM") as ps_o:
            # Deferred -1/den (3-stage Newton, one [1,1024] DVE op per stage,
            # spread across the next group's duo slots) -> gpsimd broadcast ->
            # ou multiplies. At most one group is in flight at a time.
            den_pend = []

            def den_step(p):
                st = p["stage"]
                p["stage"] += 1
                fpso = p["pso"]
                if st == 0:
                    # bit-trick seed for -1/den
                    p["sd"] = ivp.tile([1, 2, CH], f32, tag="sd", bufs=2, name="sd")
                    nc.vector.tensor_scalar(
                        out=p["sd"][:].bitcast(mybir.dt.int32),
                        in0=fpso[D:D + 1, :, :].bitcast(mybir.dt.int32),
                        scalar1=-1, scalar2=RMAGIC,
                        op0=ALU.mult, op1=ALU.add)
                elif st == 1:
                    p["tt"] = ivp.tile([1, 2, CH], f32, tag="tt", bufs=2, name="tt")
                    nc.vector.tensor_mul(p["tt"][:], p["sd"][:],
                                         fpso[D:D + 1, :, :])
                elif st == 2:
                    # one Newton step; sign folded into Wo on the host
                    p["inv"] = ivp.tile([1, 2, CH], f16, tag="iv", bufs=2, name="inv")
                    nc.vector.scalar_tensor_tensor(
                        out=p["inv"][:], in0=p["tt"][:], scalar=2.0,
                        in1=p["sd"][:], op0=ALU.subtract, op1=ALU.mult)
                elif st == 3:
                    p["bc"] = ivp.tile([D, 2, CH], f16, tag="bc", bufs=2, name="bc")
                    nc.gpsimd.partition_broadcast(p["bc"][:, :, :],
                                                  p["inv"][0:1, :, :])
                else:
                    nc.vector.tensor_mul(ou[0:D, p["pr"], p["cs"]],
                                         fpso[0:D, 0, :], p["bc"][:, 0, :])
                    nc.vector.tensor_mul(ou[D:2 * D, p["pr"], p["cs"]],
                                         fpso[0:D, 1, :], p["bc"][:, 1, :])

            def den_hook(stage_limit):
                while den_pend and den_pend[0]["stage"] < stage_limit:
                    den_step(den_pend[0])

            def den_drain():
                while den_pend:
                    p = den_pend[0]
                    while p["stage"] <= 4:
                        den_step(p)
                    den_pend.pop(0)

            def emit_phase_c(c):
                # out-projection for chunk c in 2 waves of 4 ets (each po
                # tile spans 2 PSUM banks and holds 2 et outputs)
                cs = slice(c * CH, (c + 1) * CH)
                for w in range(2):
                    po = []
                    for i in range(2):
                        po.append(ps_s.tile([P, 2, CH], f32,
                                            tag=("sa" if i == 0 else "sb"),
                                            bufs=1, name=f"po{c}_{w}_{i}"))
                    for t in range(4):
                        for i in range(2):
                            for jj in range(2):
                                et = 4 * w + 2 * i + jj
                                nc.tensor.matmul(
                                    po[i][:, jj, :],
                                    lhsT=wo_t[:, t, et * P:(et + 1) * P],
                                    rhs=ou[:, t, cs],
                                    start=(t == 0), stop=(t == 3))
                    for i in range(2):
                        for jj in range(2):
                            et = 4 * w + 2 * i + jj
                            out_t = otp.tile([P, CH], f16, tag="ot", bufs=4,
                                             name=f"ot{c}_{et}")
                            if (i + jj) % 2 == 0:
                                nc.scalar.add(out_t[:], po[i][:, jj, :],
                                              boc[:, et:et + 1])
                            else:
                                nc.vector.tensor_scalar_add(out_t[:],
                                                            po[i][:, jj, :],
                                                            boc[:, et:et + 1])
                            nc.sync.dma_start(out=yT[et * P:(et + 1) * P, cs],
                                              in_=out_t[:])

            for c in range(NCH):
                cs = slice(c * CH, (c + 1) * CH)
                for pr in range(4):
                    hA, hB = 2 * pr, 2 * pr + 1
                    pso = ps_o.tile([D + 1, 2, CH], f32, tag="o", bufs=2,
                                    name=f"pso{c}{pr}")
                    vpend = []
                    for d in range(NDUO):
                        # both heads of one st share a PSUM tile so a single
                        # exp op releases it (a split releaser desynchronizes
                        # the A/B score pairs in the scheduler)
                        pst2 = []
                        for j in range(2):
                            ps2 = ps_s.tile([P, 2, CH], f32,
                                            tag=("sa" if j == 0 else "sb"),
                                            bufs=1, name=f"ps{c}{pr}{d}{j}")
                            ks = slice((2 * d + j) * P, (2 * d + j + 1) * P)
                            nc.tensor.matmul(ps2[:, 0, :], lhsT=kT[0:D, pr, ks],
                                             rhs=qT[0:D, pr, cs],
                                             start=True, stop=True)
                            nc.tensor.matmul(ps2[:, 1, :], lhsT=kT[D:P, pr, ks],
                                             rhs=qT[D:P, pr, cs],
                                             start=True, stop=True)
                            pst2.append(ps2)
                        # exp per st (1024 free elems per op, both heads)
                        # DVE (slower op) takes the even st whose scores
                        # finish first, ScalarE the odd st: both PSUM tiles
                        # then free at ~the same time, so the next duo's four
                        # score MMs dispatch as one run (split releases cost a
                        # stationary-class transition per st).
                        ex2 = []
                        for j in range(2):
                            ex = ep.tile([P, 2, CH], f16,
                                         tag=("xa" if j == 0 else "xb"), bufs=5,
                                         name=f"ex{c}{pr}{d}{j}")
                            if j == 1 or d in (0, 2, 4):
                                # ScalarE covers the even st too while the DVE
                                # runs one Newton stage for the prev group
                                nc.scalar.activation(ex[:], pst2[j][:], AF.Exp,
                                                     scale=0.125)
                            else:
                                nc.vector.tensor_scalar(out=ex[:].bitcast(i16),
                                                        in0=pst2[j][:],
                                                        scalar1=SCH_A,
                                                        scalar2=SCH_B,
                                                        op0=ALU.mult,
                                                        op1=ALU.add)
                            ex2.append(ex)
                        vpend.append((d, ex2[0], ex2[1]))
                        if d == 0:
                            den_hook(1)      # Newton seed for prev group
                        elif d == 2:
                            den_hook(2)      # Newton multiply
                        elif d == 4:
                            den_hook(3)      # Newton refine -> -1/den f16
                        elif d == 5:
                            den_hook(4)      # gpsimd broadcast
                        elif d == 7:
                            den_hook(5)      # ou multiplies after the last
                                             # Schraudolph: the DVE never
                                             # delays a PSUM-tile release
                        if len(vpend) > LAG:
                            dd, e0, e1 = vpend.pop(0)
                            for j, ee in ((0, e0), (1, e1)):
                                pst = 2 * dd + j
                                nc.tensor.matmul(pso[:, 0, :],
                                                 lhsT=vbuf[:, pst, hA, :],
                                                 rhs=ee[:, 0, :],
                                                 start=(pst == 0), stop=False,
                                                 skip_group_check=True)
                                nc.tensor.matmul(pso[:, 1, :],
                                                 lhsT=vbuf[:, pst, hB, :],
                                                 rhs=ee[:, 1, :],
                                                 start=(pst == 0), stop=False,
                                                 skip_group_check=True)
                    for dd, e0, e1 in vpend:
                        for j, ee in ((0, e0), (1, e1)):
                            pst = 2 * dd + j
                            nc.tensor.matmul(pso[:, 0, :],
                                             lhsT=vbuf[:, pst, hA, :],
                                             rhs=ee[:, 0, :],
                                             start=False, stop=(pst == 15),
                                             skip_group_check=True)
                            nc.tensor.matmul(pso[:, 1, :],
                                             lhsT=vbuf[:, pst, hB, :],
                                             rhs=ee[:, 1, :],
                                             start=False, stop=(pst == 15),
                                             skip_group_check=True)
                    # the previous group's ou multiplies land after this
                    # group's Schraudolph stream so the DVE never delays exp
                    den_drain()
                    den_pend.append({"stage": 0, "pso": pso, "pr": pr, "cs": cs})
                    if c > 0 and pr == 0:
                        # chunk c-1's out-projection waited one full group, so
                        # its pair-3 normalization is already complete
                        emit_phase_c(c - 1)
            # final group: split the den chain by head so the gpsimd
            # broadcast of head A overlaps the DVE Newton ops of head B
            # (the [1,N] single-lane ops otherwise serialize ~6.7us with
            # nothing left to cover them)
            while den_pend:
                p = den_pend.pop(0)
                fpso, fpr, fcs = p["pso"], p["pr"], p["cs"]
                if p["stage"] != 0:
                    den_pend.insert(0, p)
                    den_drain()
                    break
                bch = []
                for h in range(2):
                    sdh = ivp.tile([1, 1, CH], f32, tag="sd", bufs=2,
                                   name=f"sdh{h}")
                    nc.vector.tensor_scalar(
                        out=sdh[:].bitcast(mybir.dt.int32),
                        in0=fpso[D:D + 1, h:h + 1, :].bitcast(mybir.dt.int32),
                        scalar1=-1, scalar2=RMAGIC,
                        op0=ALU.mult, op1=ALU.add)
                    tth = ivp.tile([1, 1, CH], f32, tag="tt", bufs=2,
                                   name=f"tth{h}")
                    nc.vector.tensor_mul(tth[:], sdh[:],
                                         fpso[D:D + 1, h:h + 1, :])
                    invh = ivp.tile([1, 1, CH], f16, tag="iv", bufs=2,
                                    name=f"invh{h}")
                    nc.vector.scalar_tensor_tensor(
                        out=invh[:], in0=tth[:], scalar=2.0,
                        in1=sdh[:], op0=ALU.subtract, op1=ALU.mult)
                    bc = ivp.tile([D, 1, CH], f16, tag="bc", bufs=2,
                                  name=f"bch{h}")
                    nc.gpsimd.partition_broadcast(bc[:, :, :],
                                                  invh[0:1, :, :])
                    bch.append(bc)
                for h in range(2):
                    nc.vector.tensor_mul(ou[h * D:(h + 1) * D, fpr, fcs],
                                         fpso[0:D, h, :], bch[h][:, 0, :])
            emit_phase_c(NCH - 1)

    nc.finalize()
    return nc


def _get_nc():
    if "nc" not in _cached:
        _cached["nc"] = _build()
    return _cached["nc"]


def _in_maps(query, key, value, Wq, bq, Wk, bk, Wv, bv, Wo, bo):
    query = np.asarray(query, np.float32)
    key = np.asarray(key, np.float32)
    value = np.asarray(value, np.float32)
    maps = []
    xT = {}
    for b in range(B):
        xT[("q", b)] = np.ascontiguousarray(query[b].T.astype(np.float16))
        xT[("k", b)] = np.ascontiguousarray(key[b].T.astype(np.float16))
        xT[("v", b)] = np.ascontiguousarray(value[b].T.astype(np.float16))
    for c in range(N_CORES):
        b, hh = divmod(c, 2)
        sl = slice(hh * HH, (hh + 1) * HH)

        def wcols(W):
            Ws = np.asarray(W, np.float32)[:, sl].astype(np.float16)
            return np.ascontiguousarray(Ws.reshape(8, P, HH).transpose(1, 0, 2))

        # negated: the kernel's normalization produces -attn (3-op Newton
        # yields -1/den), so -Wo restores the sign in the output projection
        wo_s = (-np.asarray(Wo, np.float32)[sl, :]).astype(np.float16)   # [512, E]
        wo_r = np.ascontiguousarray(wo_s.reshape(4, P, E).transpose(1, 0, 2))
        bo_c = (np.asarray(bo, np.float32).reshape(8, P).T if hh == 0
                else np.zeros((P, 8), np.float32))
        bv_b = np.ascontiguousarray(
            np.tile(np.asarray(bv, np.float32)[sl].astype(np.float16),
                    (P, 1)).reshape(P, 8, D))
        maps.append({
            "xqT": xT[("q", b)],
            "xkT": xT[("k", b)],
            "xvT": xT[("v", b)],
            "wq": wcols(Wq),
            "wk": wcols(Wk),
            "wv": wcols(Wv),
            "bq_col": np.ascontiguousarray(np.asarray(bq, np.float32)[sl].reshape(4, P).T),
            "bk_col": np.ascontiguousarray(np.asarray(bk, np.float32)[sl].reshape(4, P).T),
            "bv_bc": bv_b,
            "wo": wo_r,
            "bo_col": np.ascontiguousarray(bo_c),
        })
    return maps


def _assemble(results):
    outs = [results[c]["yT"] for c in range(N_CORES)]
    return np.stack([
        (outs[2 * b].astype(np.float32) + outs[2 * b + 1].astype(np.float32)).T
        for b in range(B)
    ]).astype(np.float32)


def kernel(**inputs):
    nc = _get_nc()
    maps = _in_maps(**inputs)
    r = run_bass_kernel_spmd(nc, maps, list(range(N_CORES)))
    return _assemble(r.results)


def _ensure_ntff_hook():
    """Register the axon NTFF profiling hook (missing antenv.axon_hooks shim)."""
    import contextlib
    import ctypes
    import types

    try:
        from antenv.axon_hooks import get_axon_ntff_profile_hook
        if get_axon_ntff_profile_hook() is not None:
            return
    except ImportError:
        pass

    import antenv

    holder = {}
    mod = types.ModuleType("antenv.axon_hooks")
    mod.set_axon_ntff_profile_hook = lambda h: holder.__setitem__("h", h)
    mod.get_axon_ntff_profile_hook = lambda: holder.get("h")
    sys.modules["antenv.axon_hooks"] = mod
    antenv.axon_hooks = mod

    so_path = "/opt/axon/libaxon_pjrt.so"
    lib = ctypes.CDLL(so_path)
    if not hasattr(lib, "axon_start_nrt_profile"):
        return
    lib.axon_start_nrt_profile.argtypes = [ctypes.POINTER(ctypes.c_int64), ctypes.c_size_t]
    lib.axon_start_nrt_profile.restype = ctypes.c_int64
    lib.axon_stop_nrt_profile.argtypes = [ctypes.c_char_p]
    lib.axon_stop_nrt_profile.restype = ctypes.c_int64

    @contextlib.contextmanager
    def _hook(output_dir, device_ids):
        import jax

        jax.devices()
        if device_ids:
            ids = (ctypes.c_int64 * len(device_ids))(*device_ids)
            rc = lib.axon_start_nrt_profile(ids, len(device_ids))
        else:
            rc = lib.axon_start_nrt_profile(None, 0)
        if rc != 0:
            raise RuntimeError(f"axon_start_nrt_profile rc={rc}")
        try:
            yield
        finally:
            n = lib.axon_stop_nrt_profile(str(output_dir).encode())
            if n < 0:
                raise RuntimeError(f"axon_stop_nrt_profile rc={n}")

    mod.set_axon_ntff_profile_hook(_hook)


def kernel_traced(tmpdir=None, **inputs):
    """Like kernel() but with NTFF tracing; returns (output, exec_time_ns)."""
    _ensure_ntff_hook()
    import concourse.bass_utils as bu
    bu.upload_artifacts = lambda d: d  # no artifact bucket in this container
    nc = _get_nc()
    maps = _in_maps(**inputs)
    r = run_bass_kernel_spmd(nc, maps, list(range(N_CORES)), trace=True, tmpdir=tmpdir)
    return _assemble(r.results), r.exec_time_ns
